# revision 8
# baseline (speedup 1.0000x reference)
"""Causal self-attention block (QKV proj -> causal attention -> out proj)
on 8 trn2 NeuronCores.

Sharding: Megatron-style. Data-parallel over batch (B=2 -> 2 groups of 4
cores), tensor-parallel over heads within a group (16 heads -> 4 heads per
core). Each core computes a partial c_proj output [T, C] for its batch;
the host sums the 4 partials per batch (the TP all-reduce) and adds b_proj.

v2 dataflow: fp8 e4m3 DoubleRow matmuls (two 128-deep k-tiles contracted
per instruction at the double-pumped fp8 rate) for the QKV projection and
the AV matmul, bf16 for S and c_proj. Causal attention gives early query
rows almost no averaging over keys, so fp8 value/logit noise there lands
directly in the output: chunk 0 (queries and keys 0..511) therefore runs
the bf16 path end-to-end (QK/V projection, exp, AV), while chunks 1-3 run
fp8 (their attention averages over >=512 keys, washing quantization noise
out). The early-key V values are fp8-quantized from the accurate bf16
projection for use by later chunks.

  - exp on ACT writes fp8 (bf16 for chunk 0) directly; causal masking of
    diagonal blocks is affine_select on GPSIMD, fully-masked column
    stripes are memset-zeroed so the DoubleRow AV can share one column
    range across its two k-tiles.
  - V is stored as [128, 2, 4 heads, 64+1] per k-tile-pair with a ones
    column per head: the softmax denominator rides row 64 of the AV
    accumulator. y^T is normalized via reciprocal_approx_fast (denominator
    row bounced p64->p0 on DVE, broadcast on GPSIMD) with the normalize
    multiply doubling as the PSUM evacuation.
  - c_proj partials stream to DRAM as bf16, deferred one chunk and
    interleaved into the next chunk's attention groups so the PE never
    idles while ACT churns through exp.
"""

import sys
import os

try:
    import concourse  # noqa: F401  (provided by the image's site path)
except ImportError:
    for _p in ("/opt/trn_rl_repo",):
        if _p not in sys.path and os.path.isdir(_p):
            sys.path.insert(0, _p)

import numpy as np

# bass_utils imports antenv.axon_hooks when tracing is requested (e.g. via
# BASS_TRACE in the environment); some images lack that module. Register a
# no-op stub so tracing degrades gracefully instead of crashing.
try:
    from antenv import axon_hooks as _ah  # noqa: F401
except ImportError:
    import types as _types
    try:
        import antenv as _antenv
        _stub = _types.ModuleType("antenv.axon_hooks")
        _stub._hook = None
        _stub.set_axon_ntff_profile_hook = (
            lambda h: setattr(_stub, "_hook", h))
        _stub.get_axon_ntff_profile_hook = lambda: _stub._hook
        sys.modules["antenv.axon_hooks"] = _stub
        _antenv.axon_hooks = _stub
    except ImportError:
        pass

import concourse.bass as bass
import concourse.bacc as bacc
import concourse.mybir as mybir
from concourse import tile
from concourse.bass_utils import run_bass_kernel_spmd

B, T, C, H = 2, 2048, 1024, 16
HD = C // H            # 64
NH = 4                 # heads per core
N_CORES = 8
P = 128
NCI = C // P           # 8 c_in tiles
NCP = NCI // 2         # 4 c_in tile PAIRS (DoubleRow granularity)
NCO = 4                # qk c_out tiles of 128 (Q01, Q23, K01, K23)
NTC = T // 512         # 4 t-chunks of 512
NTT = T // P           # 16 t-tiles of 128
F32 = mybir.dt.float32
BF16 = mybir.dt.bfloat16
F8 = mybir.dt.float8e4
DR = mybir.MatmulPerfMode.DoubleRow
SCALE = 1.0 / np.sqrt(HD)   # 0.125, folded into exp


def _trace_kernel(tc, xT, xTb, wqk, wqkb, wv, wvb, bqk, bvb, wp, out):
    from contextlib import ExitStack

    nc = tc.nc
    AF = mybir.ActivationFunctionType

    with ExitStack() as ctx:
        const = ctx.enter_context(tc.tile_pool(name="const", bufs=1))
        qkT_pool = ctx.enter_context(tc.tile_pool(name="qkTp", bufs=1))
        vv_pool = ctx.enter_context(tc.tile_pool(name="vvp", bufs=1))
        yT_pool = ctx.enter_context(tc.tile_pool(name="yTp", bufs=1))
        xt_pool = ctx.enter_context(tc.tile_pool(name="xtp", bufs=2))
        ex_pool = ctx.enter_context(tc.tile_pool(name="exp", bufs=6))
        rec_pool = ctx.enter_context(tc.tile_pool(name="recp", bufs=4))
        outs_pool = ctx.enter_context(tc.tile_pool(name="outsp", bufs=3))
        sp_pool = ctx.enter_context(tc.tile_pool(name="spp", bufs=2, space="PSUM"))
        av_pool = ctx.enter_context(tc.tile_pool(name="avp", bufs=2, space="PSUM"))
        pj_pool = ctx.enter_context(tc.tile_pool(name="pjp", bufs=2, space="PSUM"))

        # ---- persistent tiles ----
        wqk_t = [const.tile([P, 2, 512], F8, name=f"wqk{i}", tag=f"wqk{i}")
                 for i in range(NCP)]
        wv_t = [const.tile([P, 2, 256], F8, name=f"wv{i}", tag=f"wv{i}")
                for i in range(NCP)]
        wqkb_t = [const.tile([P, 512], BF16, name=f"wqkb{i}", tag=f"wqkb{i}")
                  for i in range(NCI)]
        wvb_t = [const.tile([P, 256], BF16, name=f"wvb{i}", tag=f"wvb{i}")
                 for i in range(NCI)]
        wp_t = [const.tile([P, 1024], BF16, name=f"wp{i}", tag=f"wp{i}")
                for i in range(2)]
        bqk_t = [const.tile([P, 1], F32, name=f"bqk{i}", tag=f"bqk{i}")
                 for i in range(NCO)]
        bvb_t = const.tile([P, 256], F32, name="bvb", tag="bvb")
        # qkT[0]=Q heads(0,1), [1]=Q heads(2,3), [2]=K heads(0,1), [3]=K heads(2,3)
        qkT = [qkT_pool.tile([P, T], BF16, name=f"qkT{i}", tag=f"qkT{i}")
               for i in range(NCO)]
        # vv[ktp]: fp8 V for k-tile pair ktp; per (j, head): 64 v-cols + ones,
        # zero-padded to 128 cols (dual-fp8 LDWEIGHTS requires col_grp=0xf,
        # i.e. a full 128-column weight load; the pad rows of the AV output
        # are never read and cost no matmul time)
        vv = [vv_pool.tile([P, 2, NH, P], F8, name=f"vv{t}", tag=f"vv{t}")
              for t in range(NTT // 2)]
        # vvb[kt]: bf16 V for chunk-0 k-tiles (used by chunk-0 queries)
        vvb = [vv_pool.tile([P, NH, HD + 1], BF16, name=f"vvb{t}", tag=f"vvb{t}")
               for t in range(4)]
        # yT[p]: heads (2p, 2p+1) stacked -> exactly the c_proj lhsT layout
        yT = [yT_pool.tile([P, T], BF16, name=f"yT{p}", tag=f"yT{p}")
              for p in range(2)]

        # chunk-0 bf16 x tile, loaded first
        xtb = xt_pool.tile([P, NCI, 512], BF16, name="xtb", tag="xtb")

        def load_xt_chunk(tci):
            xt = xt_pool.tile([P, NCI, 512], F8, name="xt", tag="xt")
            for ci in range(NCI):
                nc.sync.dma_start(
                    xt[:, ci, :],
                    xT[ci * P:(ci + 1) * P, tci * 512:(tci + 1) * 512])
            return xt

        # ---- load weights / biases ----
        # DMA queue order = need order: bf16 qk weights interleaved with the
        # bf16 chunk-0 x (first matmul needs wqkb[0]+xtb[0] only)
        for i in range(NCI):
            nc.sync.dma_start(wqkb_t[i][:], wqkb[i * P:(i + 1) * P, :])
            nc.sync.dma_start(xtb[:, i, :], xTb[i * P:(i + 1) * P, :])
        for i in range(NCO):
            nc.sync.dma_start(bqk_t[i][:], bqk[i])
        for i in range(NCI):
            nc.sync.dma_start(wvb_t[i][:], wvb[i * P:(i + 1) * P, :])
        nc.sync.dma_start(bvb_t[:], bvb[:])
        # fp8 weights + chunk-1 x, needed a chunk later
        for i in range(NCP):
            nc.sync.dma_start(wqk_t[i][:], wqk[i])
        xt_next = load_xt_chunk(1)
        for i in range(NCP):
            nc.sync.dma_start(wv_t[i][:], wv[i])
        for i in range(2):
            nc.sync.dma_start(wp_t[i][:], wp[i])

        # ---- ones columns (denominator trick) + zero pad ----
        for t in range(NTT // 2):
            nc.gpsimd.memset(vv[t][:, :, :, HD:HD + 1], 1.0)
            nc.gpsimd.memset(vv[t][:, :, :, HD + 1:], 0.0)
        for t in range(4):
            nc.gpsimd.memset(vvb[t][:, :, HD:HD + 1], 1.0)

        def emit_c(tg):
            # c_proj for t-tile tg; mc sequential so only one PSUM tile lives
            for mc in range(2):
                cps = pj_pool.tile([P, 512], F32, name="o_ps", tag="pj")
                for pr in range(2):
                    nc.tensor.matmul(
                        cps[:],
                        lhsT=yT[pr][:, tg * P:(tg + 1) * P],
                        rhs=wp_t[pr][:, mc * 512:(mc + 1) * 512],
                        start=(pr == 0), stop=(pr == 1))
                ot = outs_pool.tile([P, 512], BF16, name="ot", tag="ot")
                nc.vector.tensor_copy(ot[:], cps[:])
                nc.sync.dma_start(
                    out[tg * P:(tg + 1) * P, mc * 512:(mc + 1) * 512], ot[:])

        def emit_qk_proj(tci, xt):
            # Q^T / K^T: out[c_out, t], lhsT = W (stationary), rhs = xT
            for co in range(NCO):
                ps = pj_pool.tile([P, 512], F32, name="qk_ps", tag="pj")
                if tci == 0:
                    for ci in range(NCI):
                        nc.tensor.matmul(
                            ps[:],
                            lhsT=wqkb_t[ci][:, co * P:(co + 1) * P],
                            rhs=xt[:, ci, :],
                            start=(ci == 0), stop=(ci == NCI - 1))
                else:
                    for i4 in range(NCP):
                        nc.tensor.matmul(
                            ps[:],
                            lhsT=wqk_t[i4][:, :, co * P:(co + 1) * P],
                            rhs=xt[:, 2 * i4:2 * i4 + 2, :],
                            start=(i4 == 0), stop=(i4 == NCP - 1),
                            perf_mode=DR)
                nc.vector.tensor_scalar_add(
                    qkT[co][:, tci * 512:(tci + 1) * 512], ps[:],
                    bqk_t[co][:])

        def emit_v_proj(tci, xt, tt):
            # V: out[t, d], lhsT = xT tile (stationary), rhs = Wv
            tg = tci * 4 + tt
            ps = pj_pool.tile([P, 256], F32, name="v_ps", tag="pj")
            if tci == 0:
                for ci in range(NCI):
                    nc.tensor.matmul(
                        ps[:],
                        lhsT=xt[:, ci, tt * P:(tt + 1) * P],
                        rhs=wvb_t[ci][:],
                        start=(ci == 0), stop=(ci == NCI - 1))
                # accurate bf16 copy for chunk-0 queries
                nc.vector.tensor_add(
                    out=vvb[tt][:, :, 0:HD],
                    in0=ps[:].rearrange("p (h c) -> p h c", c=HD),
                    in1=bvb_t[:].rearrange("p (h c) -> p h c", c=HD))
            else:
                for i4 in range(NCP):
                    nc.tensor.matmul(
                        ps[:],
                        lhsT=xt[:, 2 * i4:2 * i4 + 2, tt * P:(tt + 1) * P],
                        rhs=wv_t[i4][:],
                        start=(i4 == 0), stop=(i4 == NCP - 1),
                        perf_mode=DR)
            # fp8 copy for later chunks' DoubleRow AV
            nc.vector.tensor_add(
                out=vv[tg // 2][:, tg % 2, :, 0:HD],
                in0=ps[:].rearrange("p (h c) -> p h c", c=HD),
                in1=bvb_t[:].rearrange("p (h c) -> p h c", c=HD))

        xts = {1: xt_next}
        for tci in range(NTC):
            # ======== projections for t-chunk tci ========
            f8c = tci > 0
            xt = xtb if tci == 0 else xts[tci]
            emit_qk_proj(tci, xt)
            for tt in range(4):
                emit_v_proj(tci, xt, tt)

            # prefetch a later chunk's xT now, so those loads sit in the
            # DMA queue ahead of this chunk's output stores (chunk 1 was
            # prefetched with the initial loads)
            if tci + 2 < NTC:
                xts[tci + 2] = load_xt_chunk(tci + 2)

            # c_proj tiles of the previous chunk, interleaved into this
            # chunk's attention groups below
            fillers = [] if tci == 0 else [(tci - 1) * 4 + tt for tt in range(4)]
            ngr = (4 * tci + 4) // 2
            steps = 2 * ngr
            fill_at = {}
            for fi, tg in enumerate(fillers):
                fill_at[(fi + 1) * steps // (len(fillers) + 1)] = tg
            step_i = [0]

            def maybe_fill():
                tg = fill_at.pop(step_i[0], None)
                if tg is not None:
                    emit_c(tg)
                step_i[0] += 1

            # ======== attention for q-chunk tci ========
            q0 = tci * 512
            nkt = 4 * tci + 4     # causal: k-tiles 0 .. 4*tci+3
            for pair in range(2):
                av = [av_pool.tile([P, 512], F32, name=f"av{u}", tag="av")
                      for u in range(2)]

                def c0_of(kt):
                    # columns q0+c0.. are the causally unmasked ones
                    return max(0, kt - 4 * tci) * P

                def emit_scores(g):
                    # S^T + exp + causal cleanup, for kts (2g, 2g+1),
                    # both heads of the pair
                    kts = (2 * g, 2 * g + 1)
                    diag = kts[1] - 4 * tci >= 0
                    exs = []
                    for u in range(2):
                        off = u * 64
                        sp = sp_pool.tile([P, 2 * 512], F32, name="sp",
                                          tag="sp")
                        ex = ex_pool.tile([P, 2 * 512], F8 if f8c else BF16,
                                          name="ex", tag="ex")
                        for j, kt in enumerate(kts):
                            c0 = c0_of(kt)
                            # K=64 on array row half `u` (concurrent rows)
                            nc.tensor.matmul(
                                sp[:, j * 512 + c0:(j + 1) * 512],
                                lhsT=qkT[2 + pair][off:off + 64,
                                                   kt * P:(kt + 1) * P],
                                rhs=qkT[pair][off:off + 64,
                                              q0 + c0:q0 + 512],
                                start=True, stop=True)
                        if diag:
                            # per-kt exp over the causally live columns,
                            # then zero the [c_av, c0(kt1)) stripe of kt1
                            # (read by the shared-range DoubleRow AV) and
                            # affine-mask the diagonal blocks on GPSIMD
                            for j, kt in enumerate(kts):
                                c0 = c0_of(kt)
                                nc.scalar.activation(
                                    ex[:, j * 512 + c0:(j + 1) * 512],
                                    sp[:, j * 512 + c0:(j + 1) * 512],
                                    AF.Exp, scale=SCALE)
                            if f8c:
                                c_av = c0_of(kts[0])
                                c1 = c0_of(kts[1])
                                if c1 > c_av:
                                    nc.gpsimd.memset(
                                        ex[:, 512 + c_av:512 + c1], 0.0)
                            for j, kt in enumerate(kts):
                                if kt - 4 * tci < 0:
                                    continue
                                b0 = j * 512 + c0_of(kt)
                                nc.gpsimd.affine_select(
                                    out=ex[:, b0:b0 + P],
                                    in_=ex[:, b0:b0 + P],
                                    compare_op=mybir.AluOpType.is_ge,
                                    fill=0.0,
                                    base=0, channel_multiplier=-1,
                                    pattern=[[1, P]],
                                )
                        else:
                            # one exp covers both kts
                            nc.scalar.activation(ex[:], sp[:],
                                                 AF.Exp, scale=SCALE)
                        exs.append(ex)
                    return g, exs

                def emit_av(g, exs, us=(0, 1)):
                    kts = (2 * g, 2 * g + 1)
                    for u in us:
                        h = 2 * pair + u
                        if f8c:
                            c_av = c0_of(kts[0])
                            nc.tensor.matmul(
                                av[u][:, c_av:512],
                                lhsT=vv[g][:, :, h, :],
                                rhs=exs[u].rearrange(
                                    "p (j n) -> p j n", j=2)[:, :, c_av:512],
                                start=(g == 0), stop=(g == nkt // 2 - 1),
                                perf_mode=DR,
                                skip_group_check=True)
                        else:
                            for j, kt in enumerate(kts):
                                c0 = c0_of(kt)
                                nc.tensor.matmul(
                                    av[u][0:HD + 1, c0:512],
                                    lhsT=vvb[kt][:, h, :],
                                    rhs=exs[u][:, j * 512 + c0:(j + 1) * 512],
                                    start=(kt == 0), stop=(kt == nkt - 1),
                                    skip_group_check=True)

                def emit_norm(u):
                    off = u * 64
                    dnm = rec_pool.tile([1, 512], F32, name="dnm", tag="dnm")
                    rc1 = rec_pool.tile([1, 512], F32, name="rc1", tag="rc1")
                    rc = rec_pool.tile([64, 512], F32, name="rc", tag="rc")
                    # denominator row (p64, PSUM) -> p0 SBUF on DVE, then
                    # fast reciprocal (needs partition-0 SBUF), broadcast
                    # on GPSIMD, and one fused normalize-evacuate multiply
                    nc.vector.tensor_copy(dnm[:], av[u][64:65, :])
                    nc.vector.reciprocal_approx_fast(rc1[:], dnm[:])
                    nc.gpsimd.partition_broadcast(rc[:], rc1[:])
                    nc.vector.tensor_mul(
                        out=yT[pair][off:off + 64, q0:q0 + 512],
                        in0=av[u][0:64, :], in1=rc[:])

                # software pipeline: scores of group g+1 issue before the
                # AVs of group g, so exp is never on the PE critical path.
                # The last group finishes per-head so each normalize chain
                # starts as early as possible.
                prev = None
                for g in range(nkt // 2):
                    cur = emit_scores(g)
                    if prev is not None:
                        emit_av(*prev)
                    maybe_fill()
                    prev = cur
                emit_av(*prev, us=(0,))
                emit_norm(0)
                emit_av(*prev, us=(1,))
                emit_norm(1)

        for tt in range(4):
            emit_c((NTC - 1) * 4 + tt)


_PROGRAM = None


def _build_program():
    global _PROGRAM
    if _PROGRAM is not None:
        return _PROGRAM
    nc = bacc.Bacc("TRN2", target_bir_lowering=False, debug=False,
                   num_devices=N_CORES)
    xT = nc.dram_tensor("xT", [C, T], F8, kind="ExternalInput").ap()
    xTb = nc.dram_tensor("xTb", [C, 512], BF16, kind="ExternalInput").ap()
    wqk = nc.dram_tensor("wqk", [NCP, P, 2, 512], F8, kind="ExternalInput").ap()
    wqkb = nc.dram_tensor("wqkb", [C, 512], BF16, kind="ExternalInput").ap()
    wv = nc.dram_tensor("wv", [NCP, P, 2, 256], F8, kind="ExternalInput").ap()
    wvb = nc.dram_tensor("wvb", [C, 256], BF16, kind="ExternalInput").ap()
    bqk = nc.dram_tensor("bqk", [NCO, P, 1], F32, kind="ExternalInput").ap()
    bvb = nc.dram_tensor("bvb", [P, NH * HD], F32, kind="ExternalInput").ap()
    wp = nc.dram_tensor("wp", [2, P, C], BF16, kind="ExternalInput").ap()
    out = nc.dram_tensor("out", [T, C], BF16, kind="ExternalOutput").ap()
    with tile.TileContext(nc) as tc:
        _trace_kernel(tc, xT, xTb, wqk, wqkb, wv, wvb, bqk, bvb, wp, out)
    nc.compile()
    _PROGRAM = nc
    return nc


def make_in_maps(x, W_attn, b_attn, W_proj):
    """Shard full inputs into the 8 per-core input maps."""
    import ml_dtypes
    bf16 = ml_dtypes.bfloat16
    f8 = mybir.dt.np(F8)
    x = np.ascontiguousarray(np.asarray(x, dtype=np.float32))
    W_attn = np.asarray(W_attn, dtype=np.float32)
    b_attn = np.asarray(b_attn, dtype=np.float32)
    W_proj = np.asarray(W_proj, dtype=np.float32)
    in_maps = []
    for cid in range(N_CORES):
        b = cid // 4
        g = cid % 4
        cs = g * NH * HD          # 256-wide head-group slice
        ce = cs + NH * HD
        xTf = x[b].T
        xT8 = np.ascontiguousarray(
            np.clip(xTf, -240.0, 240.0).astype(f8))             # [C, T]
        xTb = np.ascontiguousarray(xTf[:, 0:512].astype(bf16))  # [C, 512]
        qk_cols = np.concatenate(
            [W_attn[:, cs:ce], W_attn[:, C + cs:C + ce]], axis=1)  # [C, 512]
        wqk8 = np.ascontiguousarray(
            qk_cols.reshape(NCP, 2, P, 512).transpose(0, 2, 1, 3)
            .astype(f8))                                        # [4,128,2,512]
        wqkb = np.ascontiguousarray(qk_cols.astype(bf16))       # [C, 512]
        v_cols = W_attn[:, 2 * C + cs:2 * C + ce]               # [C, 256]
        wv8 = np.ascontiguousarray(
            v_cols.reshape(NCP, 2, P, 256).transpose(0, 2, 1, 3)
            .astype(f8))                                        # [4,128,2,256]
        wvb = np.ascontiguousarray(v_cols.astype(bf16))         # [C, 256]
        bqk = np.ascontiguousarray(
            np.concatenate([b_attn[cs:ce], b_attn[C + cs:C + ce]])
            .reshape(NCO, P, 1))
        bvb = np.ascontiguousarray(
            np.broadcast_to(b_attn[2 * C + cs:2 * C + ce], (P, NH * HD)))
        wpb = np.ascontiguousarray(
            W_proj[cs:ce, :].reshape(2, P, C).astype(bf16))     # [2,128,1024]
        in_maps.append({"xT": xT8, "xTb": xTb, "wqk": wqk8, "wqkb": wqkb,
                        "wv": wv8, "wvb": wvb, "bqk": bqk,
                        "bvb": bvb, "wp": wpb})
    return in_maps


def combine_outputs(results, b_proj):
    """Sum the TP partials per batch group and add b_proj."""
    b_proj = np.asarray(b_proj, dtype=np.float32)
    out = np.empty((B, T, C), dtype=np.float32)
    for b in range(B):
        acc = results[4 * b]["out"].astype(np.float32)
        for g in range(1, 4):
            acc += results[4 * b + g]["out"].astype(np.float32)
        out[b] = acc + b_proj[None, :]
    return out


def kernel(x, W_attn, b_attn, W_proj, b_proj, _run_kwargs=None):
    nc = _build_program()
    in_maps = make_in_maps(x, W_attn, b_attn, W_proj)
    res = run_bass_kernel_spmd(nc, in_maps, core_ids=list(range(N_CORES)),
                               **(_run_kwargs or {}))
    out = combine_outputs(res.results, b_proj)
    if _run_kwargs:
        kernel.last_results = res
    return out


if __name__ == "__main__":
    rng = np.random.default_rng(0)
    x = rng.standard_normal((B, T, C), dtype=np.float32)
    W_attn = (rng.standard_normal((C, 3 * C), dtype=np.float32) * 0.02)
    b_attn = np.zeros(3 * C, np.float32)
    W_proj = (rng.standard_normal((C, C), dtype=np.float32) * 0.02)
    b_proj = np.zeros(C, np.float32)
    out = kernel(x=x, W_attn=W_attn, b_attn=b_attn, W_proj=W_proj, b_proj=b_proj)
    print("ok", out.shape, float(np.abs(out).max()))


# revision 15
# speedup vs baseline: 1.1229x; 1.1229x over previous
"""Causal self-attention block (QKV proj -> causal attention -> out proj)
on 8 trn2 NeuronCores.

Sharding: Megatron-style. Data-parallel over batch (B=2 -> 2 groups of 4
cores), tensor-parallel over heads within a group (16 heads -> 4 heads per
core). Each core computes a partial c_proj output [T, C] for its batch;
the host sums the 4 partials per batch (the TP all-reduce) and adds b_proj.

v2 dataflow: fp8 e4m3 DoubleRow matmuls (two 128-deep k-tiles contracted
per instruction at the double-pumped fp8 rate) for the QKV projection and
the AV matmul, bf16 for S and c_proj. Causal attention gives early query
rows almost no averaging over keys, so fp8 value/logit noise there lands
directly in the output: chunk 0 (queries and keys 0..511) therefore runs
the bf16 path end-to-end (QK/V projection, exp, AV), while chunks 1-3 run
fp8 (their attention averages over >=512 keys, washing quantization noise
out). The early-key V values are fp8-quantized from the accurate bf16
projection for use by later chunks.

  - exp on ACT writes fp8 (bf16 for chunk 0) directly; causal masking of
    diagonal blocks is affine_select on GPSIMD, fully-masked column
    stripes are memset-zeroed so the DoubleRow AV can share one column
    range across its two k-tiles.
  - V is stored as [128, 2, 4 heads, 64+1] per k-tile-pair with a ones
    column per head: the softmax denominator rides row 64 of the AV
    accumulator. y^T is normalized via reciprocal_approx_fast (denominator
    row bounced p64->p0 on DVE, broadcast on GPSIMD) with the normalize
    multiply doubling as the PSUM evacuation.
  - c_proj partials stream to DRAM as bf16, deferred one chunk and
    interleaved into the next chunk's attention groups so the PE never
    idles while ACT churns through exp.
"""

import sys
import os

try:
    import concourse  # noqa: F401  (provided by the image's site path)
except ImportError:
    for _p in ("/opt/trn_rl_repo",):
        if _p not in sys.path and os.path.isdir(_p):
            sys.path.insert(0, _p)

import numpy as np

# bass_utils imports antenv.axon_hooks when tracing is requested (e.g. via
# BASS_TRACE in the environment); some images lack that module. Register a
# no-op stub so tracing degrades gracefully instead of crashing.
try:
    from antenv import axon_hooks as _ah  # noqa: F401
except ImportError:
    import types as _types
    try:
        import antenv as _antenv
        _stub = _types.ModuleType("antenv.axon_hooks")
        _stub._hook = None
        _stub.set_axon_ntff_profile_hook = (
            lambda h: setattr(_stub, "_hook", h))
        _stub.get_axon_ntff_profile_hook = lambda: _stub._hook
        sys.modules["antenv.axon_hooks"] = _stub
        _antenv.axon_hooks = _stub
    except ImportError:
        pass

import concourse.bass as bass
import concourse.bacc as bacc
import concourse.mybir as mybir
from concourse import tile
from concourse.bass_utils import run_bass_kernel_spmd

B, T, C, H = 2, 2048, 1024, 16
HD = C // H            # 64
NH = 4                 # heads per core
N_CORES = 8
P = 128
NCI = C // P           # 8 c_in tiles
NCP = NCI // 2         # 4 c_in tile PAIRS (DoubleRow granularity)
NCO = 4                # qk c_out tiles of 128 (Q01, Q23, K01, K23)
NTC = T // 512         # 4 t-chunks of 512
NTT = T // P           # 16 t-tiles of 128
F32 = mybir.dt.float32
BF16 = mybir.dt.bfloat16
F8 = mybir.dt.float8e4
DR = mybir.MatmulPerfMode.DoubleRow
SCALE = 1.0 / np.sqrt(HD)   # 0.125, folded into exp


def _trace_kernel(tc, xT, xTb, wqk, wqkb, wv, wvb, bqk, bvb, wp, out):
    from contextlib import ExitStack

    nc = tc.nc
    AF = mybir.ActivationFunctionType

    with ExitStack() as ctx:
        const = ctx.enter_context(tc.tile_pool(name="const", bufs=1))
        qkT_pool = ctx.enter_context(tc.tile_pool(name="qkTp", bufs=1))
        vv_pool = ctx.enter_context(tc.tile_pool(name="vvp", bufs=1))
        yT_pool = ctx.enter_context(tc.tile_pool(name="yTp", bufs=1))
        xt_pool = ctx.enter_context(tc.tile_pool(name="xtp", bufs=2))
        ex_pool = ctx.enter_context(tc.tile_pool(name="exp", bufs=6))
        rec_pool = ctx.enter_context(tc.tile_pool(name="recp", bufs=4))
        outs_pool = ctx.enter_context(tc.tile_pool(name="outsp", bufs=3))
        sp_pool = ctx.enter_context(tc.tile_pool(name="spp", bufs=2, space="PSUM"))
        av_pool = ctx.enter_context(tc.tile_pool(name="avp", bufs=2, space="PSUM"))
        pj_pool = ctx.enter_context(tc.tile_pool(name="pjp", bufs=2, space="PSUM"))

        # ---- persistent tiles ----
        wqk_t = [const.tile([P, 2, 512], F8, name=f"wqk{i}", tag=f"wqk{i}")
                 for i in range(NCP)]
        wv_t = [const.tile([P, 2, 256], F8, name=f"wv{i}", tag=f"wv{i}")
                for i in range(NCP)]
        wqkb_t = [const.tile([P, 512], BF16, name=f"wqkb{i}", tag=f"wqkb{i}")
                  for i in range(NCI)]
        wvb_t = [const.tile([P, 256], BF16, name=f"wvb{i}", tag=f"wvb{i}")
                 for i in range(NCI)]
        wp_t = [const.tile([P, 1024], BF16, name=f"wp{i}", tag=f"wp{i}")
                for i in range(2)]
        bqk_t = [const.tile([P, 1], F32, name=f"bqk{i}", tag=f"bqk{i}")
                 for i in range(NCO)]
        bvb_t = const.tile([P, 256], F32, name="bvb", tag="bvb")
        # qkT[0]=Q heads(0,1), [1]=Q heads(2,3), [2]=K heads(0,1), [3]=K heads(2,3)
        qkT = [qkT_pool.tile([P, T], BF16, name=f"qkT{i}", tag=f"qkT{i}")
               for i in range(NCO)]
        # vv[ktp]: fp8 V for k-tile pair ktp; per (j, head): 64 v-cols + ones,
        # zero-padded to 128 cols (dual-fp8 LDWEIGHTS requires col_grp=0xf,
        # i.e. a full 128-column weight load; the pad rows of the AV output
        # are never read and cost no matmul time)
        vv = [vv_pool.tile([P, 2, NH, P], F8, name=f"vv{t}", tag=f"vv{t}")
              for t in range(NTT // 2)]
        # vvb[kt]: bf16 V for chunk-0 k-tiles (used by chunk-0 queries)
        vvb = [vv_pool.tile([P, NH, HD + 1], BF16, name=f"vvb{t}", tag=f"vvb{t}")
               for t in range(4)]
        # yT[p]: heads (2p, 2p+1) stacked -> exactly the c_proj lhsT layout
        yT = [yT_pool.tile([P, T], BF16, name=f"yT{p}", tag=f"yT{p}")
              for p in range(2)]

        # chunk-0 bf16 x tile, loaded first
        xtb = xt_pool.tile([P, NCI, 512], BF16, name="xtb", tag="xtb")

        def load_xt_chunk(tci):
            xt = xt_pool.tile([P, NCI, 512], F8, name="xt", tag="xt")
            for ci in range(NCI):
                nc.sync.dma_start(
                    xt[:, ci, :],
                    xT[ci * P:(ci + 1) * P, tci * 512:(tci + 1) * 512])
            return xt

        # ---- load weights / biases ----
        # DMA queue order = need order: bf16 qk weights interleaved with the
        # bf16 chunk-0 x (first matmul needs wqkb[0]+xtb[0] only)
        for i in range(NCI):
            nc.sync.dma_start(wqkb_t[i][:], wqkb[i * P:(i + 1) * P, :])
            nc.sync.dma_start(xtb[:, i, :], xTb[i * P:(i + 1) * P, :])
        for i in range(NCO):
            nc.sync.dma_start(bqk_t[i][:], bqk[i])
        for i in range(NCI):
            nc.sync.dma_start(wvb_t[i][:], wvb[i * P:(i + 1) * P, :])
        nc.sync.dma_start(bvb_t[:], bvb[:])
        # wp before the fp8 weights: the first c_proj fillers fire early in
        # chunk-1's attention and stall the whole in-order PE queue if wp
        # hasn't landed
        for i in range(2):
            nc.sync.dma_start(wp_t[i][:], wp[i])
        # fp8 weights + chunk-1 x, needed a chunk later
        for i in range(NCP):
            nc.sync.dma_start(wqk_t[i][:], wqk[i])
        xt_next = load_xt_chunk(1)
        for i in range(NCP):
            nc.sync.dma_start(wv_t[i][:], wv[i])

        # ---- ones columns (denominator trick) + zero pad ----
        for t in range(NTT // 2):
            nc.gpsimd.memset(vv[t][:, :, :, HD:HD + 1], 1.0)
            nc.gpsimd.memset(vv[t][:, :, :, HD + 1:], 0.0)
        for t in range(4):
            nc.gpsimd.memset(vvb[t][:, :, HD:HD + 1], 1.0)

        def emit_c_mc(tg, mc):
            # half a c_proj t-tile; one PSUM bank, one filler-sized unit
            cps = pj_pool.tile([P, 512], F32, name="o_ps", tag="pj")
            for pr in range(2):
                nc.tensor.matmul(
                    cps[:],
                    lhsT=yT[pr][:, tg * P:(tg + 1) * P],
                    rhs=wp_t[pr][:, mc * 512:(mc + 1) * 512],
                    start=(pr == 0), stop=(pr == 1))
            ot = outs_pool.tile([P, 512], BF16, name="ot", tag="ot")
            nc.vector.tensor_copy(ot[:], cps[:])
            nc.sync.dma_start(
                out[tg * P:(tg + 1) * P, mc * 512:(mc + 1) * 512], ot[:])

        def emit_qk_co(tci, xt, co):
            # Q^T / K^T: out[c_out, t], lhsT = W (stationary), rhs = xT
            ps = pj_pool.tile([P, 512], F32, name="qk_ps", tag="pj")
            if tci == 0:
                for ci in range(NCI):
                    nc.tensor.matmul(
                        ps[:],
                        lhsT=wqkb_t[ci][:, co * P:(co + 1) * P],
                        rhs=xt[:, ci, :],
                        start=(ci == 0), stop=(ci == NCI - 1))
            else:
                for i4 in range(NCP):
                    nc.tensor.matmul(
                        ps[:],
                        lhsT=wqk_t[i4][:, :, co * P:(co + 1) * P],
                        rhs=xt[:, 2 * i4:2 * i4 + 2, :],
                        start=(i4 == 0), stop=(i4 == NCP - 1),
                        perf_mode=DR)
            nc.vector.tensor_scalar_add(
                qkT[co][:, tci * 512:(tci + 1) * 512], ps[:],
                bqk_t[co][:])

        def emit_v_proj(tci, xt, tt):
            # V: out[t, d], lhsT = xT tile (stationary), rhs = Wv
            tg = tci * 4 + tt
            ps = pj_pool.tile([P, 256], F32, name="v_ps", tag="pj")
            if tci == 0:
                for ci in range(NCI):
                    nc.tensor.matmul(
                        ps[:],
                        lhsT=xt[:, ci, tt * P:(tt + 1) * P],
                        rhs=wvb_t[ci][:],
                        start=(ci == 0), stop=(ci == NCI - 1))
                # accurate bf16 copy for chunk-0 queries
                nc.vector.tensor_add(
                    out=vvb[tt][:, :, 0:HD],
                    in0=ps[:].rearrange("p (h c) -> p h c", c=HD),
                    in1=bvb_t[:].rearrange("p (h c) -> p h c", c=HD))
            else:
                for i4 in range(NCP):
                    nc.tensor.matmul(
                        ps[:],
                        lhsT=xt[:, 2 * i4:2 * i4 + 2, tt * P:(tt + 1) * P],
                        rhs=wv_t[i4][:],
                        start=(i4 == 0), stop=(i4 == NCP - 1),
                        perf_mode=DR)
            # fp8 copy for later chunks' DoubleRow AV
            nc.vector.tensor_add(
                out=vv[tg // 2][:, tg % 2, :, 0:HD],
                in0=ps[:].rearrange("p (h c) -> p h c", c=HD),
                in1=bvb_t[:].rearrange("p (h c) -> p h c", c=HD))

        xts = {1: xt_next}

        def proj_units(tci):
            xt = xtb if tci == 0 else xts[tci]
            us = [(lambda co=co: emit_qk_co(tci, xt, co)) for co in range(NCO)]
            us += [(lambda tt=tt: emit_v_proj(tci, xt, tt)) for tt in range(4)]
            return us

        def cproj_units(tci):
            return [(lambda tg=tg, mc=mc: emit_c_mc(tg, mc))
                    for tg in range(tci * 4, tci * 4 + 4) for mc in range(2)]

        # attention per-group steps are ACT(exp)-bound; these PE-dense units
        # are dispensed between group steps so the PE pipe never drains.
        # proj of chunk t+1 weaves into chunk t's attention; c_proj partials
        # are deferred to the exp-heaviest late chunks.
        fill_plan = {
            0: lambda: proj_units(1),
            1: lambda: proj_units(2),
            2: lambda: proj_units(3) + cproj_units(0),
            3: lambda: cproj_units(1) + cproj_units(2),
        }

        for tci in range(NTC):
            # ======== projections for t-chunk tci ========
            f8c = tci > 0
            if tci == 0:
                for u in proj_units(0):
                    u()

            # prefetch a later chunk's xT now, so those loads sit in the
            # DMA queue ahead of this chunk's output stores (chunk 1 was
            # prefetched with the initial loads)
            if tci + 2 < NTC:
                xts[tci + 2] = load_xt_chunk(tci + 2)

            fillers = fill_plan[tci]()
            ngr = (4 * tci + 4) // 2
            steps = 2 * ngr
            step_i = [0]
            filled = [0]

            def maybe_fill():
                step_i[0] += 1
                want = len(fillers) * step_i[0] // steps
                while filled[0] < want:
                    fillers[filled[0]]()
                    filled[0] += 1

            # ======== attention for q-chunk tci ========
            q0 = tci * 512
            nkt = 4 * tci + 4     # causal: k-tiles 0 .. 4*tci+3
            for pair in range(2):
                av = [av_pool.tile([P, 512], F32, name=f"av{u}", tag="av")
                      for u in range(2)]

                def c0_of(kt):
                    # columns q0+c0.. are the causally unmasked ones
                    return max(0, kt - 4 * tci) * P

                def emit_scores(g):
                    # S^T + exp + causal cleanup, for kts (2g, 2g+1),
                    # both heads of the pair
                    kts = (2 * g, 2 * g + 1)
                    diag = kts[1] - 4 * tci >= 0
                    exs = []
                    for u in range(2):
                        off = u * 64
                        sp = sp_pool.tile([P, 2 * 512], F32, name="sp",
                                          tag="sp")
                        ex = ex_pool.tile([P, 2 * 512], F8 if f8c else BF16,
                                          name="ex", tag="ex")
                        for j, kt in enumerate(kts):
                            c0 = c0_of(kt)
                            # K=64 on array row half `u` (concurrent rows)
                            nc.tensor.matmul(
                                sp[:, j * 512 + c0:(j + 1) * 512],
                                lhsT=qkT[2 + pair][off:off + 64,
                                                   kt * P:(kt + 1) * P],
                                rhs=qkT[pair][off:off + 64,
                                              q0 + c0:q0 + 512],
                                start=True, stop=True)
                        if diag:
                            # per-kt exp over the causally live columns,
                            # then zero the [c_av, c0(kt1)) stripe of kt1
                            # (read by the shared-range DoubleRow AV) and
                            # affine-mask the diagonal blocks on GPSIMD
                            for j, kt in enumerate(kts):
                                c0 = c0_of(kt)
                                nc.scalar.activation(
                                    ex[:, j * 512 + c0:(j + 1) * 512],
                                    sp[:, j * 512 + c0:(j + 1) * 512],
                                    AF.Exp, scale=SCALE)
                            if f8c:
                                c_av = c0_of(kts[0])
                                c1 = c0_of(kts[1])
                                if c1 > c_av:
                                    nc.gpsimd.memset(
                                        ex[:, 512 + c_av:512 + c1], 0.0)
                            for j, kt in enumerate(kts):
                                if kt - 4 * tci < 0:
                                    continue
                                b0 = j * 512 + c0_of(kt)
                                nc.gpsimd.affine_select(
                                    out=ex[:, b0:b0 + P],
                                    in_=ex[:, b0:b0 + P],
                                    compare_op=mybir.AluOpType.is_ge,
                                    fill=0.0,
                                    base=0, channel_multiplier=-1,
                                    pattern=[[1, P]],
                                )
                        else:
                            # one exp covers both kts
                            nc.scalar.activation(ex[:], sp[:],
                                                 AF.Exp, scale=SCALE)
                        exs.append(ex)
                    return g, exs

                def emit_av(g, exs, us=(0, 1)):
                    kts = (2 * g, 2 * g + 1)
                    for u in us:
                        h = 2 * pair + u
                        if f8c:
                            c_av = c0_of(kts[0])
                            nc.tensor.matmul(
                                av[u][:, c_av:512],
                                lhsT=vv[g][:, :, h, :],
                                rhs=exs[u].rearrange(
                                    "p (j n) -> p j n", j=2)[:, :, c_av:512],
                                start=(g == 0), stop=(g == nkt // 2 - 1),
                                perf_mode=DR,
                                skip_group_check=True)
                        else:
                            for j, kt in enumerate(kts):
                                c0 = c0_of(kt)
                                nc.tensor.matmul(
                                    av[u][0:HD + 1, c0:512],
                                    lhsT=vvb[kt][:, h, :],
                                    rhs=exs[u][:, j * 512 + c0:(j + 1) * 512],
                                    start=(kt == 0), stop=(kt == nkt - 1),
                                    skip_group_check=True)

                def emit_norms():
                    # denominator row (p64, PSUM) -> p0 SBUF on DVE, fast
                    # reciprocal (needs partition-0 SBUF), broadcast on
                    # GPSIMD, and a fused normalize-evacuate multiply
                    for u in range(2):
                        dnm = rec_pool.tile([1, 512], F32, name="dnm", tag="dnm")
                        rc1 = rec_pool.tile([1, 512], F32, name="rc1", tag="rc1")
                        rc = rec_pool.tile([64, 512], F32, name="rc", tag="rc")
                        nc.vector.tensor_copy(dnm[:], av[u][64:65, :])
                        nc.vector.reciprocal_approx_fast(rc1[:], dnm[:])
                        nc.gpsimd.partition_broadcast(rc[:], rc1[:])
                        nc.vector.tensor_mul(
                            out=yT[pair][u * 64:u * 64 + 64, q0:q0 + 512],
                            in0=av[u][0:64, :], in1=rc[:])

                # software pipeline: scores of group g+1 issue before the
                # AVs of group g, so exp is never on the PE critical path.
                # The last group finishes per-head so each normalize chain
                # starts as early as possible.
                prev = None
                for g in range(nkt // 2):
                    cur = emit_scores(g)
                    if prev is not None:
                        emit_av(*prev)
                    maybe_fill()
                    prev = cur
                emit_av(*prev)
                emit_norms()

        for u in cproj_units(NTC - 1):
            u()


_PROGRAM = None


def _build_program():
    global _PROGRAM
    if _PROGRAM is not None:
        return _PROGRAM
    nc = bacc.Bacc("TRN2", target_bir_lowering=False, debug=False,
                   num_devices=N_CORES)
    xT = nc.dram_tensor("xT", [C, T], F8, kind="ExternalInput").ap()
    xTb = nc.dram_tensor("xTb", [C, 512], BF16, kind="ExternalInput").ap()
    wqk = nc.dram_tensor("wqk", [NCP, P, 2, 512], F8, kind="ExternalInput").ap()
    wqkb = nc.dram_tensor("wqkb", [C, 512], BF16, kind="ExternalInput").ap()
    wv = nc.dram_tensor("wv", [NCP, P, 2, 256], F8, kind="ExternalInput").ap()
    wvb = nc.dram_tensor("wvb", [C, 256], BF16, kind="ExternalInput").ap()
    bqk = nc.dram_tensor("bqk", [NCO, P, 1], F32, kind="ExternalInput").ap()
    bvb = nc.dram_tensor("bvb", [P, NH * HD], F32, kind="ExternalInput").ap()
    wp = nc.dram_tensor("wp", [2, P, C], BF16, kind="ExternalInput").ap()
    out = nc.dram_tensor("out", [T, C], BF16, kind="ExternalOutput").ap()
    with tile.TileContext(nc) as tc:
        _trace_kernel(tc, xT, xTb, wqk, wqkb, wv, wvb, bqk, bvb, wp, out)
    nc.compile()
    _PROGRAM = nc
    return nc


def make_in_maps(x, W_attn, b_attn, W_proj):
    """Shard full inputs into the 8 per-core input maps."""
    import ml_dtypes
    bf16 = ml_dtypes.bfloat16
    f8 = mybir.dt.np(F8)
    x = np.ascontiguousarray(np.asarray(x, dtype=np.float32))
    W_attn = np.asarray(W_attn, dtype=np.float32)
    b_attn = np.asarray(b_attn, dtype=np.float32)
    W_proj = np.asarray(W_proj, dtype=np.float32)
    in_maps = []
    for cid in range(N_CORES):
        b = cid // 4
        g = cid % 4
        cs = g * NH * HD          # 256-wide head-group slice
        ce = cs + NH * HD
        xTf = x[b].T
        xT8 = np.ascontiguousarray(
            np.clip(xTf, -240.0, 240.0).astype(f8))             # [C, T]
        xTb = np.ascontiguousarray(xTf[:, 0:512].astype(bf16))  # [C, 512]
        qk_cols = np.concatenate(
            [W_attn[:, cs:ce], W_attn[:, C + cs:C + ce]], axis=1)  # [C, 512]
        wqk8 = np.ascontiguousarray(
            qk_cols.reshape(NCP, 2, P, 512).transpose(0, 2, 1, 3)
            .astype(f8))                                        # [4,128,2,512]
        wqkb = np.ascontiguousarray(qk_cols.astype(bf16))       # [C, 512]
        v_cols = W_attn[:, 2 * C + cs:2 * C + ce]               # [C, 256]
        wv8 = np.ascontiguousarray(
            v_cols.reshape(NCP, 2, P, 256).transpose(0, 2, 1, 3)
            .astype(f8))                                        # [4,128,2,256]
        wvb = np.ascontiguousarray(v_cols.astype(bf16))         # [C, 256]
        bqk = np.ascontiguousarray(
            np.concatenate([b_attn[cs:ce], b_attn[C + cs:C + ce]])
            .reshape(NCO, P, 1))
        bvb = np.ascontiguousarray(
            np.broadcast_to(b_attn[2 * C + cs:2 * C + ce], (P, NH * HD)))
        wpb = np.ascontiguousarray(
            W_proj[cs:ce, :].reshape(2, P, C).astype(bf16))     # [2,128,1024]
        in_maps.append({"xT": xT8, "xTb": xTb, "wqk": wqk8, "wqkb": wqkb,
                        "wv": wv8, "wvb": wvb, "bqk": bqk,
                        "bvb": bvb, "wp": wpb})
    return in_maps


def combine_outputs(results, b_proj):
    """Sum the TP partials per batch group and add b_proj."""
    b_proj = np.asarray(b_proj, dtype=np.float32)
    out = np.empty((B, T, C), dtype=np.float32)
    for b in range(B):
        acc = results[4 * b]["out"].astype(np.float32)
        for g in range(1, 4):
            acc += results[4 * b + g]["out"].astype(np.float32)
        out[b] = acc + b_proj[None, :]
    return out


def kernel(x, W_attn, b_attn, W_proj, b_proj, _run_kwargs=None):
    nc = _build_program()
    in_maps = make_in_maps(x, W_attn, b_attn, W_proj)
    res = run_bass_kernel_spmd(nc, in_maps, core_ids=list(range(N_CORES)),
                               **(_run_kwargs or {}))
    out = combine_outputs(res.results, b_proj)
    if _run_kwargs:
        kernel.last_results = res
    return out


if __name__ == "__main__":
    rng = np.random.default_rng(0)
    x = rng.standard_normal((B, T, C), dtype=np.float32)
    W_attn = (rng.standard_normal((C, 3 * C), dtype=np.float32) * 0.02)
    b_attn = np.zeros(3 * C, np.float32)
    W_proj = (rng.standard_normal((C, C), dtype=np.float32) * 0.02)
    b_proj = np.zeros(C, np.float32)
    out = kernel(x=x, W_attn=W_attn, b_attn=b_attn, W_proj=W_proj, b_proj=b_proj)
    print("ok", out.shape, float(np.abs(out).max()))
